# revision 4
# baseline (speedup 1.0000x reference)
"""Self-contained TRN2 Bass kernel for nn_MultiHeadAttn_91010357002583.

Multi-head attention (B=4, S=2048, D=1024, H=16, hd=64), eval mode,
mask all-ones, char_ids/seq_len unused by the reference.

Sharding: 8 cores = 4 batches x 2 query-row halves. Each core:
  - receives x^T (bf16) for its batch with ITS query half's rows FIRST
    (attention is permutation-invariant over key rows, so reordering
    k rows is free; q rows stay in original order within the half);
  - computes full K^T / V for the batch (2x redundant) + Q^T for its half;
  - flash-style attention with scores transposed [k, q], softmax
    denominator fused as a col-tiled all-ones stationary matmul;
  - fc projection on its disjoint 1024 output rows.
Output is a pure concatenation — no collectives, no host reduction.
"""

import math
import sys
from contextlib import ExitStack

import numpy as np
import ml_dtypes

for _p in ("/opt/trn_rl_repo", "/root/.axon_site/_ro/trn_rl_repo"):
    if _p not in sys.path:
        sys.path.insert(0, _p)

import concourse.bass as bass  # noqa: E402
import concourse.tile as tile  # noqa: E402
from concourse import bacc, mybir  # noqa: E402
from concourse.bass_utils import run_bass_kernel_spmd  # noqa: E402

bf16 = ml_dtypes.bfloat16
FP32 = mybir.dt.float32
BF16 = mybir.dt.bfloat16
AF = mybir.ActivationFunctionType

B, S, D, H = 4, 2048, 1024, 16
HD = D // H
SCALE = math.sqrt(HD)


class Cfg:
    def __init__(self, R=2048, Q=1024, Hn=16, D=1024, repeats=1,
                 attn_scheme="coltile", interleave=True, no_ones=False,
                 norm_mode="direct", abufs=3, dbufs=1):
        assert R % 128 == 0 and Q % 128 == 0 and Hn % 4 == 0
        self.R, self.Q, self.Hn, self.D = R, Q, Hn, D
        self.FT = D // 128          # feature tiles (proj contraction)
        self.NCT = Hn // 2          # coltiles for Q (and K) = heads/2
        self.NRT = R // 128         # k row tiles
        self.NG = Hn // 4           # head groups of 4
        self.NJ = Hn * 64 // 128    # d-tiles for fc contraction
        self.NQT = Q // 128
        self.NRC = max(1, R // 1024)
        self.repeats = repeats
        self.attn_scheme = attn_scheme
        self.interleave = interleave
        self.no_ones = no_ones
        self.norm_mode = norm_mode
        self.abufs = abufs
        self.dbufs = dbufs
        self.scale = 1.0 / math.sqrt(64.0)


def build_nc(cfg: Cfg, num_devices=8):
    R, Q, Hn, Dm, FT = cfg.R, cfg.Q, cfg.Hn, cfg.D, cfg.FT
    nc = bacc.Bacc("TRN2", target_bir_lowering=False, debug=False,
                   enable_asserts=False, num_devices=num_devices)
    xt_d = nc.dram_tensor("xt", [Dm, R], BF16, kind="ExternalInput").ap()
    wqk_d = nc.dram_tensor("wqk", [Hn, 128, FT, 128], BF16,
                           kind="ExternalInput").ap()
    wv_d = nc.dram_tensor("wv", [FT, 128, Hn * 64], BF16,
                          kind="ExternalInput").ap()
    wfc_d = nc.dram_tensor("wfc", [cfg.NJ, 128, Dm], BF16,
                           kind="ExternalInput").ap()
    bfc_d = nc.dram_tensor("bfc", [128, Dm], FP32, kind="ExternalInput").ap()
    y_d = nc.dram_tensor("y", [Q, Dm], FP32, kind="ExternalOutput").ap()
    with tile.TileContext(nc) as tc:
        with ExitStack() as ctx:
            build_body(ctx, tc, cfg, xt_d, wqk_d, wv_d, wfc_d, bfc_d, y_d)
    nc.finalize()
    return nc


def build_body(ctx, tc, cfg: Cfg, xt_d, wqk_d, wv_d, wfc_d, bfc_d, y_d):
    nc = tc.nc
    R, Q, Hn, Dm, FT = cfg.R, cfg.Q, cfg.Hn, cfg.D, cfg.FT
    NCT, NRT, NG, NJ, NQT, NRC = (cfg.NCT, cfg.NRT, cfg.NG, cfg.NJ,
                                  cfg.NQT, cfg.NRC)
    m65 = cfg.attn_scheme == "m65"

    persist = ctx.enter_context(tc.tile_pool(name="persist", bufs=1))
    wqk_pool = ctx.enter_context(tc.tile_pool(name="wqk", bufs=6))
    wv_pool = ctx.enter_context(tc.tile_pool(name="wv", bufs=2))
    attn_pool = ctx.enter_context(tc.tile_pool(name="attn", bufs=cfg.abufs))
    ysb_pool = ctx.enter_context(tc.tile_pool(name="ysb", bufs=2))
    den_pool = ctx.enter_context(tc.tile_pool(name="den", bufs=cfg.dbufs))
    spool = ctx.enter_context(tc.tile_pool(name="ps_s", bufs=2, space="PSUM"))
    opool = ctx.enter_context(tc.tile_pool(name="ps_o", bufs=2, space="PSUM"))

    for _rep in range(cfg.repeats):
        fused = cfg.attn_scheme == "fused"
        xt_sb = persist.tile([128, FT, R], BF16, tag="xt")
        KT_sb = persist.tile([128, NCT, R], BF16, tag="kt")
        QT_sb = persist.tile([128, NCT, Q], BF16, tag="qt")
        if fused:
            # per head-pair hp: cols 0:64 = V_A, 64:128 = ones, 128:192 = V_B.
            # weight window A = [:, 0:128] -> out = [num_A; den_A],
            # weight window B = [:, 64:192] -> out = [den_B; num_B].
            V_sb = persist.tile([128, NRT, NCT, 192], BF16, tag="v")
            nc.vector.memset(V_sb[:, :, :, 64:128], 1.0)
        else:
            VW = 65 if m65 else 64
            V_sb = persist.tile([128, NRT, Hn, VW], BF16, tag="v")
        OT_sb = persist.tile([128, NJ, Q], BF16, tag="ot")
        wfc_sb = persist.tile([128, NJ, Dm], BF16, tag="wfc")
        bfc_sb = persist.tile([128, Dm], FP32, tag="bfc")
        if m65:
            ones1_sb = persist.tile([1, 64], BF16, tag="ones1")
            nc.vector.memset(ones1_sb[:], 1.0)
            nc.vector.memset(V_sb[:, :, :, 64:65], 1.0)
        elif not fused:
            ones_sb = persist.tile([128, 64], BF16, tag="ones")
            nc.vector.memset(ones_sb[:], 1.0)

        for ft in range(FT):
            nc.sync.dma_start(xt_sb[:, ft, :], xt_d[ft * 128:(ft + 1) * 128, :])

        wqk_tiles = {}

        def load_wqk(j):
            t = wqk_pool.tile([128, FT, 128], BF16, tag="wqk", name="wqk_t")
            nc.sync.dma_start(t[:], wqk_d[j])
            wqk_tiles[j] = t

        def emit_qkv_units(g):
            """Return a list of closures, each emitting one psum-chain of
            group g's QKV projection work."""
            cts = [2 * g, 2 * g + 1]
            units = []

            def load_w(g=g, cts=cts):
                wv_sb = wv_pool.tile([128, FT, 256], BF16, tag="wv",
                                     name="wv_t")
                nc.sync.dma_start(
                    wv_sb[:],
                    wv_d[:, :, g * 256:(g + 1) * 256].rearrange(
                        "f p c -> p f c"))
                for ct in cts:
                    load_wqk(ct)
                    load_wqk(NCT + ct)
                return wv_sb

            state = {}

            def ensure_w():
                if "wv" not in state:
                    state["wv"] = load_w()

            def q_unit(ct):
                def emit():
                    ensure_w()
                    ps = spool.tile([128, 1024], FP32, tag="ps_s",
                                    name="ps_q")[:, :Q]
                    for sc in range(0, Q, 512):
                        sn = min(512, Q - sc)
                        for ft in range(FT):
                            nc.tensor.matmul(
                                ps[:, sc:sc + sn], wqk_tiles[ct][:, ft, :],
                                xt_sb[:, ft, sc:sc + sn],
                                start=(ft == 0), stop=(ft == FT - 1))
                    nc.vector.tensor_copy(QT_sb[:, ct, :], ps)
                return emit

            def k_unit(ct, rc):
                def emit():
                    ensure_w()
                    rn = min(1024, R - rc * 1024)
                    ps = spool.tile([128, 1024], FP32, tag="ps_s",
                                    name="ps_k")[:, :rn]
                    for sc in range(0, rn, 512):
                        sn = min(512, rn - sc)
                        for ft in range(FT):
                            nc.tensor.matmul(
                                ps[:, sc:sc + sn],
                                wqk_tiles[NCT + ct][:, ft, :],
                                xt_sb[:, ft, rc * 1024 + sc:rc * 1024 + sc + sn],
                                start=(ft == 0), stop=(ft == FT - 1))
                    nc.vector.tensor_copy(
                        KT_sb[:, ct, rc * 1024:rc * 1024 + rn], ps)
                return emit

            def v_unit(rt):
                def emit():
                    ensure_w()
                    ps = spool.tile([128, 1024], FP32, tag="ps_s",
                                    name="ps_v")[:, :256]
                    for ft in range(FT):
                        nc.tensor.matmul(
                            ps, xt_sb[:, ft, rt * 128:(rt + 1) * 128],
                            state["wv"][:, ft, :],
                            start=(ft == 0), stop=(ft == FT - 1))
                    if cfg.attn_scheme == "fused":
                        psr = ps.rearrange("p (h c) -> p h c", c=64)
                        for h in range(4):
                            nc.vector.tensor_copy(
                                V_sb[:, rt, 2 * g + h // 2,
                                     (h % 2) * 128:(h % 2) * 128 + 64],
                                psr[:, h, :])
                    else:
                        nc.vector.tensor_copy(
                            V_sb[:, rt, 4 * g:4 * g + 4, 0:64],
                            ps.rearrange("p (h c) -> p h c", c=64))
                return emit

            for ct in cts:
                units.append(q_unit(ct))
                units.append(k_unit(ct, 0))
                if NRC > 1:
                    units.append(k_unit(ct, 1))
            for rt in range(NRT):
                units.append(v_unit(rt))
            return units

        def norm_and_store(o_, hp, half):
            """divide numerator rows by the fused denominator, write OT."""
            if m65:
                # copy psum out immediately to free the accumulator slot
                st = den_pool.tile([65, 1024], FP32, tag="st",
                                   name="st")[:, :Q]
                nc.vector.tensor_copy(st, o_[0:65, :])
                rcp = den_pool.tile([1, 1024], FP32, tag="rcp",
                                    name="rcp")[:, :Q]
                nc.vector.reciprocal(rcp, st[64:65, :])
                # bf16 hi/lo split so the bf16 broadcast matmul is exact
                hi = den_pool.tile([1, 1024], BF16, tag="rhi",
                                   name="rhi")[:, :Q]
                lo = den_pool.tile([1, 1024], BF16, tag="rlo",
                                   name="rlo")[:, :Q]
                tmp = den_pool.tile([1, 1024], FP32, tag="rtmp",
                                    name="rtmp")[:, :Q]
                nc.vector.tensor_copy(hi, rcp)
                nc.vector.tensor_tensor(tmp, rcp, hi,
                                        mybir.AluOpType.subtract)
                nc.vector.tensor_copy(lo, tmp)
                bc = spool.tile([128, 1024], FP32, tag="ps_s",
                                name="bc")[0:64, :Q]
                for sc in range(0, Q, 512):
                    sn = min(512, Q - sc)
                    s_ = slice(sc, sc + sn)
                    nc.tensor.matmul(bc[:, s_], ones1_sb[:], hi[:, s_],
                                     start=True, stop=False)
                    nc.tensor.matmul(bc[:, s_], ones1_sb[:], lo[:, s_],
                                     start=False, stop=True)
                nc.vector.tensor_mul(OT_sb[half * 64:half * 64 + 64, hp, :],
                                     st[0:64, :], bc)
            elif cfg.no_ones:
                nc.vector.tensor_copy(OT_sb[half * 64:half * 64 + 64, hp, :],
                                      o_[0:64, :])
            elif cfg.norm_mode == "copyout":
                st = den_pool.tile([128, 1024], FP32, tag="stc",
                                   name="stc")[:, :Q]
                nc.vector.tensor_copy(st, o_[:, :])
                den = den_pool.tile([64, 1024], FP32, tag="den",
                                    name="den")[:, :Q]
                nc.vector.reciprocal(den, st[64:128, :])
                nc.vector.tensor_mul(OT_sb[half * 64:half * 64 + 64, hp, :],
                                     st[0:64, :], den)
            else:
                den = den_pool.tile([64, 1024], FP32, tag="den",
                                    name="den")[:, :Q]
                nc.vector.reciprocal(den, o_[64:128, :])
                nc.vector.tensor_mul(OT_sb[half * 64:half * 64 + 64, hp, :],
                                     o_[0:64, :], den)

        def attn_step(hp, kt, oA, oB):
            psA = spool.tile([128, 1024], FP32, tag="ps_s", name="psA")[:, :Q]
            psB = spool.tile([128, 1024], FP32, tag="ps_s", name="psB")[:, :Q]
            for sc in range(0, Q, 512):
                sn = min(512, Q - sc)
                nc.tensor.matmul(
                    psA[:, sc:sc + sn],
                    KT_sb[0:64, hp, kt * 128:(kt + 1) * 128],
                    QT_sb[0:64, hp, sc:sc + sn], start=True, stop=True)
                nc.tensor.matmul(
                    psB[:, sc:sc + sn],
                    KT_sb[64:128, hp, kt * 128:(kt + 1) * 128],
                    QT_sb[64:128, hp, sc:sc + sn], start=True, stop=True)
            aA = attn_pool.tile([128, Q], BF16, tag="aT", name="aA")
            aB = attn_pool.tile([128, Q], BF16, tag="aT", name="aB")
            nc.scalar.activation(aA[:], psA, AF.Exp, scale=cfg.scale)
            nc.scalar.activation(aB[:], psB, AF.Exp, scale=cfg.scale)
            st, sp = (kt == 0), (kt == NRT - 1)
            for sc in range(0, Q, 512):
                sn = min(512, Q - sc)
                s_ = slice(sc, sc + sn)
                if cfg.attn_scheme == "fused":
                    nc.tensor.matmul(oA[:, s_], V_sb[:, kt, hp, 0:128],
                                     aA[:, s_], start=st, stop=sp)
                    nc.tensor.matmul(oB[:, s_], V_sb[:, kt, hp, 64:192],
                                     aB[:, s_], start=st, stop=sp)
                elif m65:
                    nc.tensor.matmul(oA[0:65, s_], V_sb[:, kt, 2 * hp, :],
                                     aA[:, s_], start=st, stop=sp)
                    nc.tensor.matmul(oB[0:65, s_], V_sb[:, kt, 2 * hp + 1, :],
                                     aB[:, s_], start=st, stop=sp)
                elif cfg.attn_scheme == "swap":
                    # complementary col-groups across tiles: V_A(grp0,oA) ||
                    # V_B(grp64,oB), then ones_A(grp64,oA) || ones_B(grp0,oB).
                    # Per-tile writer order identical to the safe layout.
                    nc.tensor.matmul(oA[0:64, s_],
                                     V_sb[:, kt, 2 * hp, 0:64],
                                     aA[:, s_], start=st, stop=sp)
                    nc.tensor.matmul(oB[64:128, s_],
                                     V_sb[:, kt, 2 * hp + 1, 0:64],
                                     aB[:, s_], start=st, stop=sp)
                    nc.tensor.matmul(oA[64:128, s_], ones_sb[:],
                                     aA[:, s_], start=st, stop=sp)
                    nc.tensor.matmul(oB[0:64, s_], ones_sb[:],
                                     aB[:, s_], start=st, stop=sp)
                else:
                    nc.tensor.matmul(oA[0:64, s_],
                                     V_sb[:, kt, 2 * hp, 0:64],
                                     aA[:, s_], start=st, stop=sp)
                    if not cfg.no_ones:
                        nc.tensor.matmul(oA[64:128, s_], ones_sb[:],
                                         aA[:, s_], start=st, stop=sp)
                    nc.tensor.matmul(oB[0:64, s_],
                                     V_sb[:, kt, 2 * hp + 1, 0:64],
                                     aB[:, s_], start=st, stop=sp)
                    if not cfg.no_ones:
                        nc.tensor.matmul(oB[64:128, s_], ones_sb[:],
                                         aB[:, s_], start=st, stop=sp)

        # ---- main loop: group attention with next group's QKV interleaved
        units = emit_qkv_units(0)
        for u in units:
            u()
        # fc weights aren't needed until the tail — keep them off the
        # critical-path DMA window at kernel start
        nc.sync.dma_start(wfc_sb[:], wfc_d.rearrange("j p d -> p j d"))
        nc.sync.dma_start(bfc_sb[:], bfc_d[:])
        for g in range(NG):
            next_units = emit_qkv_units(g + 1) if g + 1 < NG else []
            steps = [(hp, kt) for hp in (2 * g, 2 * g + 1)
                     for kt in range(NRT)]
            o_tiles = {}
            ui = 0
            for si, (hp, kt) in enumerate(steps):
                if kt == 0:
                    o_tiles[hp] = (
                        opool.tile([128, 1024], FP32, tag="ps_o",
                                   name="oA")[:, :Q],
                        opool.tile([128, 1024], FP32, tag="ps_o",
                                   name="oB")[:, :Q])
                oA, oB = o_tiles[hp]
                attn_step(hp, kt, oA, oB)
                if cfg.interleave:
                    want = (si + 1) * len(next_units) // len(steps)
                    while ui < want:
                        next_units[ui]()
                        ui += 1
                if kt == NRT - 1:
                    norm_and_store(oA, hp, 0)
                    if cfg.attn_scheme == "swap":
                        den = den_pool.tile([64, 1024], FP32, tag="den",
                                            name="denS")[:, :Q]
                        nc.vector.reciprocal(den, oB[0:64, :])
                        nc.vector.tensor_mul(OT_sb[64:128, hp, :],
                                             oB[64:128, :], den)
                    else:
                        norm_and_store(oB, hp, 1)
            while ui < len(next_units):
                next_units[ui]()
                ui += 1
            if not cfg.interleave:
                pass

        # ---- fc ----
        for qt in range(NQT):
            ps = spool.tile([128, 1024], FP32, tag="ps_s",
                            name="ps_fc")[:, :Dm]
            for cc in range(0, Dm, 512):
                for j in range(NJ):
                    nc.tensor.matmul(
                        ps[:, cc:cc + 512],
                        OT_sb[:, j, qt * 128:(qt + 1) * 128],
                        wfc_sb[:, j, cc:cc + 512],
                        start=(j == 0), stop=(j == NJ - 1))
            yt = ysb_pool.tile([128, Dm], FP32, tag="y", name="yt")
            nc.vector.tensor_add(yt[:], ps, bfc_sb[:])
            nc.sync.dma_start(y_d[qt * 128:(qt + 1) * 128, :], yt[:])


# ---------------- host side ----------------

def prep_core_inputs(cfg: Cfg, xb_perm, W_qkv, W_fc, b_fc):
    """xb_perm: [R, D] f32, rows already permuted (this core's q rows first)."""
    Dm, Hn, FT, NCT, NJ = cfg.D, cfg.Hn, cfg.FT, cfg.NCT, cfg.NJ
    xt = np.ascontiguousarray(xb_perm.T).astype(bf16)
    Wq = W_qkv[:, :NCT * 128]
    Wk = W_qkv[:, Dm:Dm + NCT * 128]
    Wv = W_qkv[:, 2 * Dm:2 * Dm + Hn * 64]
    wq_t = Wq.reshape(FT, 128, NCT, 128).transpose(2, 1, 0, 3)
    wk_t = Wk.reshape(FT, 128, NCT, 128).transpose(2, 1, 0, 3)
    wqk = np.ascontiguousarray(
        np.concatenate([wq_t, wk_t], axis=0)).astype(bf16)
    wv = np.ascontiguousarray(Wv.reshape(FT, 128, Hn * 64)).astype(bf16)
    wfc = np.ascontiguousarray(
        W_fc[:NJ * 128].reshape(NJ, 128, Dm)).astype(bf16)
    bfc = np.ascontiguousarray(
        np.broadcast_to(b_fc.astype(np.float32), (128, Dm)))
    return {"xt": xt, "wqk": wqk, "wv": wv, "wfc": wfc, "bfc": bfc}


_CACHE = {}


def _get_nc(repeats=1):
    key = ("nc", repeats)
    if key not in _CACHE:
        _CACHE[key] = build_nc(Cfg(R=S, Q=S // 2, Hn=H, D=D, repeats=repeats))
    return _CACHE[key]


def make_in_maps(x, W_qkv, W_fc, b_fc):
    cfg = Cfg(R=S, Q=S // 2, Hn=H, D=D)
    x = np.asarray(x, dtype=np.float32)
    in_maps = []
    for c in range(8):
        b, half = divmod(c, 2)
        r0 = half * (S // 2)
        order = np.concatenate([
            np.arange(r0, r0 + S // 2),
            np.arange(0, r0),
            np.arange(r0 + S // 2, S),
        ])
        xb = x[b][order]
        in_maps.append(prep_core_inputs(
            cfg, xb, np.asarray(W_qkv, np.float32),
            np.asarray(W_fc, np.float32), np.asarray(b_fc, np.float32)))
    return in_maps


def kernel(x, char_ids, seq_len, mask, W_qkv, W_fc, b_fc):
    """Full inputs in, full [B, S, D] float32 output out."""
    import os
    # the axon NTFF trace hook is unavailable in this container; make sure
    # an inherited BASS_TRACE=1 cannot send us down that (crashing) path
    os.environ["BASS_NEVER_TRACE"] = "1"
    nc = _get_nc(repeats=1)
    in_maps = make_in_maps(x, W_qkv, W_fc, b_fc)
    res = run_bass_kernel_spmd(nc, in_maps, core_ids=list(range(8)))
    out = np.empty((B, S, D), dtype=np.float32)
    for c in range(8):
        b, half = divmod(c, 2)
        r0 = half * (S // 2)
        out[b, r0:r0 + S // 2, :] = res.results[c]["y"]
    return out



# revision 6
# speedup vs baseline: 14.8004x; 14.8004x over previous
"""Self-contained TRN2 Bass kernel for nn_MultiHeadAttn_91010357002583.

Multi-head attention (B=4, S=2048, D=1024, H=16, hd=64), eval mode,
mask all-ones, char_ids/seq_len unused by the reference.

Sharding: 8 cores = 4 batches x 2 query-row halves. Each core:
  - receives x^T (bf16) for its batch with ITS query half's rows FIRST
    (attention is permutation-invariant over key rows, so reordering
    k rows is free; q rows stay in original order within the half);
  - computes full K^T / V for the batch (2x redundant) + Q^T for its half;
  - flash-style attention with scores transposed [k, q], softmax
    denominator fused as a col-tiled all-ones stationary matmul;
  - fc projection on its disjoint 1024 output rows.
Output is a pure concatenation — no collectives, no host reduction.
"""

import math
import sys
from contextlib import ExitStack

import numpy as np
import ml_dtypes

for _p in ("/opt/trn_rl_repo", "/root/.axon_site/_ro/trn_rl_repo"):
    if _p not in sys.path:
        sys.path.insert(0, _p)

import concourse.bass as bass  # noqa: E402
import concourse.tile as tile  # noqa: E402
from concourse import bacc, mybir  # noqa: E402
from concourse.bass_utils import run_bass_kernel_spmd  # noqa: E402

bf16 = ml_dtypes.bfloat16
FP32 = mybir.dt.float32
BF16 = mybir.dt.bfloat16
AF = mybir.ActivationFunctionType

B, S, D, H = 4, 2048, 1024, 16
HD = D // H
SCALE = math.sqrt(HD)


class Cfg:
    def __init__(self, R=2048, Q=1024, Hn=16, D=1024, repeats=1,
                 attn_scheme="coltile", interleave=True, no_ones=False,
                 norm_mode="direct", abufs=3, dbufs=1):
        assert R % 128 == 0 and Q % 128 == 0 and Hn % 4 == 0
        self.R, self.Q, self.Hn, self.D = R, Q, Hn, D
        self.FT = D // 128          # feature tiles (proj contraction)
        self.NCT = Hn // 2          # coltiles for Q (and K) = heads/2
        self.NRT = R // 128         # k row tiles
        self.NG = Hn // 4           # head groups of 4
        self.NJ = Hn * 64 // 128    # d-tiles for fc contraction
        self.NQT = Q // 128
        self.NRC = max(1, R // 1024)
        self.repeats = repeats
        self.attn_scheme = attn_scheme
        self.interleave = interleave
        self.no_ones = no_ones
        self.norm_mode = norm_mode
        self.abufs = abufs
        self.dbufs = dbufs
        self.scale = 1.0 / math.sqrt(64.0)


def build_nc(cfg: Cfg, num_devices=8):
    R, Q, Hn, Dm, FT = cfg.R, cfg.Q, cfg.Hn, cfg.D, cfg.FT
    nc = bacc.Bacc("TRN2", target_bir_lowering=False, debug=False,
                   enable_asserts=False, num_devices=num_devices)
    xt_d = nc.dram_tensor("xt", [Dm, R], BF16, kind="ExternalInput").ap()
    wqk_d = nc.dram_tensor("wqk", [Hn, 128, FT, 128], BF16,
                           kind="ExternalInput").ap()
    wv_d = nc.dram_tensor("wv", [FT, 128, Hn * 64], BF16,
                          kind="ExternalInput").ap()
    wfc_d = nc.dram_tensor("wfc", [cfg.NJ, 128, Dm], BF16,
                           kind="ExternalInput").ap()
    bfc_d = nc.dram_tensor("bfc", [128, Dm], FP32, kind="ExternalInput").ap()
    y_d = nc.dram_tensor("y", [Q, Dm], FP32, kind="ExternalOutput").ap()
    with tile.TileContext(nc) as tc:
        with ExitStack() as ctx:
            build_body(ctx, tc, cfg, xt_d, wqk_d, wv_d, wfc_d, bfc_d, y_d)
    nc.finalize()
    return nc


def build_body(ctx, tc, cfg: Cfg, xt_d, wqk_d, wv_d, wfc_d, bfc_d, y_d):
    nc = tc.nc
    R, Q, Hn, Dm, FT = cfg.R, cfg.Q, cfg.Hn, cfg.D, cfg.FT
    NCT, NRT, NG, NJ, NQT, NRC = (cfg.NCT, cfg.NRT, cfg.NG, cfg.NJ,
                                  cfg.NQT, cfg.NRC)
    m65 = cfg.attn_scheme == "m65"

    persist = ctx.enter_context(tc.tile_pool(name="persist", bufs=1))
    wqk_pool = ctx.enter_context(tc.tile_pool(name="wqk", bufs=6))
    wv_pool = ctx.enter_context(tc.tile_pool(name="wv", bufs=2))
    attn_pool = ctx.enter_context(tc.tile_pool(name="attn", bufs=cfg.abufs))
    ysb_pool = ctx.enter_context(tc.tile_pool(name="ysb", bufs=2))
    den_pool = ctx.enter_context(tc.tile_pool(name="den", bufs=cfg.dbufs))
    spool = ctx.enter_context(tc.tile_pool(name="ps_s", bufs=2, space="PSUM"))
    opool = ctx.enter_context(tc.tile_pool(name="ps_o", bufs=2, space="PSUM"))

    for _rep in range(cfg.repeats):
        fused = cfg.attn_scheme == "fused"
        xt_sb = persist.tile([128, FT, R], BF16, tag="xt")
        KT_sb = persist.tile([128, NCT, R], BF16, tag="kt")
        QT_sb = persist.tile([128, NCT, Q], BF16, tag="qt")
        if fused:
            # per head-pair hp: cols 0:64 = V_A, 64:128 = ones, 128:192 = V_B.
            # weight window A = [:, 0:128] -> out = [num_A; den_A],
            # weight window B = [:, 64:192] -> out = [den_B; num_B].
            V_sb = persist.tile([128, NRT, NCT, 192], BF16, tag="v")
            nc.vector.memset(V_sb[:, :, :, 64:128], 1.0)
        else:
            VW = 65 if m65 else 64
            V_sb = persist.tile([128, NRT, Hn, VW], BF16, tag="v")
        OT_sb = persist.tile([128, NJ, Q], BF16, tag="ot")
        wfc_sb = persist.tile([128, NJ, Dm], BF16, tag="wfc")
        bfc_sb = persist.tile([128, Dm], FP32, tag="bfc")
        if m65:
            ones1_sb = persist.tile([1, 64], BF16, tag="ones1")
            nc.vector.memset(ones1_sb[:], 1.0)
            nc.vector.memset(V_sb[:, :, :, 64:65], 1.0)
        elif not fused:
            ones_sb = persist.tile([128, 64], BF16, tag="ones")
            nc.vector.memset(ones_sb[:], 1.0)

        for ft in range(FT):
            nc.sync.dma_start(xt_sb[:, ft, :], xt_d[ft * 128:(ft + 1) * 128, :])

        wqk_tiles = {}

        def load_wqk(j):
            t = wqk_pool.tile([128, FT, 128], BF16, tag="wqk", name="wqk_t")
            nc.sync.dma_start(t[:], wqk_d[j])
            wqk_tiles[j] = t

        def emit_qkv_units(g):
            """Return a list of closures, each emitting one psum-chain of
            group g's QKV projection work."""
            cts = [2 * g, 2 * g + 1]
            units = []

            def load_w(g=g, cts=cts):
                wv_sb = wv_pool.tile([128, FT, 256], BF16, tag="wv",
                                     name="wv_t")
                nc.sync.dma_start(
                    wv_sb[:],
                    wv_d[:, :, g * 256:(g + 1) * 256].rearrange(
                        "f p c -> p f c"))
                for ct in cts:
                    load_wqk(ct)
                    load_wqk(NCT + ct)
                return wv_sb

            state = {}

            def ensure_w():
                if "wv" not in state:
                    state["wv"] = load_w()

            def q_unit(ct):
                def emit():
                    ensure_w()
                    ps = spool.tile([128, 1024], FP32, tag="ps_s",
                                    name="ps_q")[:, :Q]
                    for sc in range(0, Q, 512):
                        sn = min(512, Q - sc)
                        for ft in range(FT):
                            nc.tensor.matmul(
                                ps[:, sc:sc + sn], wqk_tiles[ct][:, ft, :],
                                xt_sb[:, ft, sc:sc + sn],
                                start=(ft == 0), stop=(ft == FT - 1))
                    nc.vector.tensor_copy(QT_sb[:, ct, :], ps)
                return emit

            def k_unit(ct, rc):
                def emit():
                    ensure_w()
                    rn = min(1024, R - rc * 1024)
                    ps = spool.tile([128, 1024], FP32, tag="ps_s",
                                    name="ps_k")[:, :rn]
                    for sc in range(0, rn, 512):
                        sn = min(512, rn - sc)
                        for ft in range(FT):
                            nc.tensor.matmul(
                                ps[:, sc:sc + sn],
                                wqk_tiles[NCT + ct][:, ft, :],
                                xt_sb[:, ft, rc * 1024 + sc:rc * 1024 + sc + sn],
                                start=(ft == 0), stop=(ft == FT - 1))
                    nc.vector.tensor_copy(
                        KT_sb[:, ct, rc * 1024:rc * 1024 + rn], ps)
                return emit

            def v_unit(rt):
                def emit():
                    ensure_w()
                    ps = spool.tile([128, 1024], FP32, tag="ps_s",
                                    name="ps_v")[:, :256]
                    for ft in range(FT):
                        nc.tensor.matmul(
                            ps, xt_sb[:, ft, rt * 128:(rt + 1) * 128],
                            state["wv"][:, ft, :],
                            start=(ft == 0), stop=(ft == FT - 1))
                    if cfg.attn_scheme == "fused":
                        psr = ps.rearrange("p (h c) -> p h c", c=64)
                        for h in range(4):
                            nc.vector.tensor_copy(
                                V_sb[:, rt, 2 * g + h // 2,
                                     (h % 2) * 128:(h % 2) * 128 + 64],
                                psr[:, h, :])
                    else:
                        nc.vector.tensor_copy(
                            V_sb[:, rt, 4 * g:4 * g + 4, 0:64],
                            ps.rearrange("p (h c) -> p h c", c=64))
                return emit

            for ct in cts:
                units.append(q_unit(ct))
                units.append(k_unit(ct, 0))
                if NRC > 1:
                    units.append(k_unit(ct, 1))
            for rt in range(NRT):
                units.append(v_unit(rt))
            return units

        def norm_and_store(o_, hp, half):
            """divide numerator rows by the fused denominator, write OT."""
            if m65:
                # copy psum out immediately to free the accumulator slot
                st = den_pool.tile([65, 1024], FP32, tag="st",
                                   name="st")[:, :Q]
                nc.vector.tensor_copy(st, o_[0:65, :])
                rcp = den_pool.tile([1, 1024], FP32, tag="rcp",
                                    name="rcp")[:, :Q]
                nc.vector.reciprocal(rcp, st[64:65, :])
                # bf16 hi/lo split so the bf16 broadcast matmul is exact
                hi = den_pool.tile([1, 1024], BF16, tag="rhi",
                                   name="rhi")[:, :Q]
                lo = den_pool.tile([1, 1024], BF16, tag="rlo",
                                   name="rlo")[:, :Q]
                tmp = den_pool.tile([1, 1024], FP32, tag="rtmp",
                                    name="rtmp")[:, :Q]
                nc.vector.tensor_copy(hi, rcp)
                nc.vector.tensor_tensor(tmp, rcp, hi,
                                        mybir.AluOpType.subtract)
                nc.vector.tensor_copy(lo, tmp)
                bc = spool.tile([128, 1024], FP32, tag="ps_s",
                                name="bc")[0:64, :Q]
                for sc in range(0, Q, 512):
                    sn = min(512, Q - sc)
                    s_ = slice(sc, sc + sn)
                    nc.tensor.matmul(bc[:, s_], ones1_sb[:], hi[:, s_],
                                     start=True, stop=False)
                    nc.tensor.matmul(bc[:, s_], ones1_sb[:], lo[:, s_],
                                     start=False, stop=True)
                nc.vector.tensor_mul(OT_sb[half * 64:half * 64 + 64, hp, :],
                                     st[0:64, :], bc)
            elif cfg.no_ones:
                nc.vector.tensor_copy(OT_sb[half * 64:half * 64 + 64, hp, :],
                                      o_[0:64, :])
            elif cfg.norm_mode == "copyout":
                st = den_pool.tile([128, 1024], FP32, tag="stc",
                                   name="stc")[:, :Q]
                nc.vector.tensor_copy(st, o_[:, :])
                den = den_pool.tile([64, 1024], FP32, tag="den",
                                    name="den")[:, :Q]
                nc.vector.reciprocal(den, st[64:128, :])
                nc.vector.tensor_mul(OT_sb[half * 64:half * 64 + 64, hp, :],
                                     st[0:64, :], den)
            else:
                den = den_pool.tile([64, 1024], FP32, tag="den",
                                    name="den")[:, :Q]
                nc.vector.reciprocal(den, o_[64:128, :])
                nc.vector.tensor_mul(OT_sb[half * 64:half * 64 + 64, hp, :],
                                     o_[0:64, :], den)

        def attn_step(hp, kt, oA, oB):
            psA = spool.tile([128, 1024], FP32, tag="ps_s", name="psA")[:, :Q]
            psB = spool.tile([128, 1024], FP32, tag="ps_s", name="psB")[:, :Q]
            for sc in range(0, Q, 512):
                sn = min(512, Q - sc)
                nc.tensor.matmul(
                    psA[:, sc:sc + sn],
                    KT_sb[0:64, hp, kt * 128:(kt + 1) * 128],
                    QT_sb[0:64, hp, sc:sc + sn], start=True, stop=True)
                nc.tensor.matmul(
                    psB[:, sc:sc + sn],
                    KT_sb[64:128, hp, kt * 128:(kt + 1) * 128],
                    QT_sb[64:128, hp, sc:sc + sn], start=True, stop=True)
            aA = attn_pool.tile([128, Q], BF16, tag="aT", name="aA")
            aB = attn_pool.tile([128, Q], BF16, tag="aT", name="aB")
            nc.scalar.activation(aA[:], psA, AF.Exp, scale=cfg.scale)
            nc.scalar.activation(aB[:], psB, AF.Exp, scale=cfg.scale)
            st, sp = (kt == 0), (kt == NRT - 1)
            for sc in range(0, Q, 512):
                sn = min(512, Q - sc)
                s_ = slice(sc, sc + sn)
                if cfg.attn_scheme == "fused":
                    nc.tensor.matmul(oA[:, s_], V_sb[:, kt, hp, 0:128],
                                     aA[:, s_], start=st, stop=sp)
                    nc.tensor.matmul(oB[:, s_], V_sb[:, kt, hp, 64:192],
                                     aB[:, s_], start=st, stop=sp)
                elif m65:
                    nc.tensor.matmul(oA[0:65, s_], V_sb[:, kt, 2 * hp, :],
                                     aA[:, s_], start=st, stop=sp)
                    nc.tensor.matmul(oB[0:65, s_], V_sb[:, kt, 2 * hp + 1, :],
                                     aB[:, s_], start=st, stop=sp)
                elif cfg.attn_scheme == "swap":
                    # complementary col-groups across tiles: V_A(grp0,oA) ||
                    # V_B(grp64,oB), then ones_A(grp64,oA) || ones_B(grp0,oB).
                    # Per-tile writer order identical to the safe layout.
                    nc.tensor.matmul(oA[0:64, s_],
                                     V_sb[:, kt, 2 * hp, 0:64],
                                     aA[:, s_], start=st, stop=sp)
                    nc.tensor.matmul(oB[64:128, s_],
                                     V_sb[:, kt, 2 * hp + 1, 0:64],
                                     aB[:, s_], start=st, stop=sp)
                    nc.tensor.matmul(oA[64:128, s_], ones_sb[:],
                                     aA[:, s_], start=st, stop=sp)
                    nc.tensor.matmul(oB[0:64, s_], ones_sb[:],
                                     aB[:, s_], start=st, stop=sp)
                else:
                    nc.tensor.matmul(oA[0:64, s_],
                                     V_sb[:, kt, 2 * hp, 0:64],
                                     aA[:, s_], start=st, stop=sp)
                    if not cfg.no_ones:
                        nc.tensor.matmul(oA[64:128, s_], ones_sb[:],
                                         aA[:, s_], start=st, stop=sp)
                    nc.tensor.matmul(oB[0:64, s_],
                                     V_sb[:, kt, 2 * hp + 1, 0:64],
                                     aB[:, s_], start=st, stop=sp)
                    if not cfg.no_ones:
                        nc.tensor.matmul(oB[64:128, s_], ones_sb[:],
                                         aB[:, s_], start=st, stop=sp)

        # ---- main loop: group attention with next group's QKV interleaved
        units = emit_qkv_units(0)
        for u in units:
            u()
        # fc weights aren't needed until the tail — keep them off the
        # critical-path DMA window at kernel start
        nc.sync.dma_start(wfc_sb[:], wfc_d.rearrange("j p d -> p j d"))
        nc.sync.dma_start(bfc_sb[:], bfc_d[:])
        for g in range(NG):
            next_units = emit_qkv_units(g + 1) if g + 1 < NG else []
            steps = [(hp, kt) for hp in (2 * g, 2 * g + 1)
                     for kt in range(NRT)]
            o_tiles = {}
            ui = 0
            for si, (hp, kt) in enumerate(steps):
                if kt == 0:
                    o_tiles[hp] = (
                        opool.tile([128, 1024], FP32, tag="ps_o",
                                   name="oA")[:, :Q],
                        opool.tile([128, 1024], FP32, tag="ps_o",
                                   name="oB")[:, :Q])
                oA, oB = o_tiles[hp]
                attn_step(hp, kt, oA, oB)
                if cfg.interleave:
                    want = (si + 1) * len(next_units) // len(steps)
                    while ui < want:
                        next_units[ui]()
                        ui += 1
                if kt == NRT - 1:
                    norm_and_store(oA, hp, 0)
                    if cfg.attn_scheme in ("swap", "fused"):
                        den = den_pool.tile([64, 1024], FP32, tag="den",
                                            name="denS")[:, :Q]
                        nc.vector.reciprocal(den, oB[0:64, :])
                        nc.vector.tensor_mul(OT_sb[64:128, hp, :],
                                             oB[64:128, :], den)
                    else:
                        norm_and_store(oB, hp, 1)
            while ui < len(next_units):
                next_units[ui]()
                ui += 1
            if not cfg.interleave:
                pass

        # ---- fc ----
        for qt in range(NQT):
            ps = spool.tile([128, 1024], FP32, tag="ps_s",
                            name="ps_fc")[:, :Dm]
            for cc in range(0, Dm, 512):
                for j in range(NJ):
                    nc.tensor.matmul(
                        ps[:, cc:cc + 512],
                        OT_sb[:, j, qt * 128:(qt + 1) * 128],
                        wfc_sb[:, j, cc:cc + 512],
                        start=(j == 0), stop=(j == NJ - 1))
            yt = ysb_pool.tile([128, Dm], FP32, tag="y", name="yt")
            nc.vector.tensor_add(yt[:], ps, bfc_sb[:])
            nc.sync.dma_start(y_d[qt * 128:(qt + 1) * 128, :], yt[:])


# ---------------- host side ----------------

def prep_core_inputs(cfg: Cfg, xb_perm, W_qkv, W_fc, b_fc):
    """xb_perm: [R, D] f32, rows already permuted (this core's q rows first)."""
    Dm, Hn, FT, NCT, NJ = cfg.D, cfg.Hn, cfg.FT, cfg.NCT, cfg.NJ
    xt = np.ascontiguousarray(xb_perm.T).astype(bf16)
    Wq = W_qkv[:, :NCT * 128]
    Wk = W_qkv[:, Dm:Dm + NCT * 128]
    Wv = W_qkv[:, 2 * Dm:2 * Dm + Hn * 64]
    wq_t = Wq.reshape(FT, 128, NCT, 128).transpose(2, 1, 0, 3)
    wk_t = Wk.reshape(FT, 128, NCT, 128).transpose(2, 1, 0, 3)
    wqk = np.ascontiguousarray(
        np.concatenate([wq_t, wk_t], axis=0)).astype(bf16)
    wv = np.ascontiguousarray(Wv.reshape(FT, 128, Hn * 64)).astype(bf16)
    wfc = np.ascontiguousarray(
        W_fc[:NJ * 128].reshape(NJ, 128, Dm)).astype(bf16)
    bfc = np.ascontiguousarray(
        np.broadcast_to(b_fc.astype(np.float32), (128, Dm)))
    return {"xt": xt, "wqk": wqk, "wv": wv, "wfc": wfc, "bfc": bfc}


_CACHE = {}

SCHEME = "coltile"


def _get_nc(repeats=1):
    key = ("nc", repeats, SCHEME)
    if key not in _CACHE:
        _CACHE[key] = build_nc(Cfg(R=S, Q=S // 2, Hn=H, D=D, repeats=repeats,
                                   attn_scheme=SCHEME))
    return _CACHE[key]


def make_in_maps(x, W_qkv, W_fc, b_fc):
    cfg = Cfg(R=S, Q=S // 2, Hn=H, D=D)
    x = np.asarray(x, dtype=np.float32)
    in_maps = []
    for c in range(8):
        b, half = divmod(c, 2)
        r0 = half * (S // 2)
        order = np.concatenate([
            np.arange(r0, r0 + S // 2),
            np.arange(0, r0),
            np.arange(r0 + S // 2, S),
        ])
        xb = x[b][order]
        in_maps.append(prep_core_inputs(
            cfg, xb, np.asarray(W_qkv, np.float32),
            np.asarray(W_fc, np.float32), np.asarray(b_fc, np.float32)))
    return in_maps


def kernel(x, char_ids, seq_len, mask, W_qkv, W_fc, b_fc):
    """Full inputs in, full [B, S, D] float32 output out."""
    import os
    # the axon NTFF trace hook is unavailable in this container; make sure
    # an inherited BASS_TRACE=1 cannot send us down that (crashing) path
    os.environ["BASS_NEVER_TRACE"] = "1"
    nc = _get_nc(repeats=1)
    in_maps = make_in_maps(x, W_qkv, W_fc, b_fc)
    res = run_bass_kernel_spmd(nc, in_maps, core_ids=list(range(8)))
    out = np.empty((B, S, D), dtype=np.float32)
    for c in range(8):
        b, half = divmod(c, 2)
        r0 = half * (S // 2)
        out[b, r0:r0 + S // 2, :] = res.results[c]["y"]
    return out



# revision 28
# speedup vs baseline: 25.1884x; 1.7019x over previous
"""Self-contained TRN2 Bass kernel for nn_MultiHeadAttn_91010357002583.

Multi-head attention (B=4, S=2048, D=1024, H=16, hd=64), eval mode,
mask all-ones, char_ids/seq_len unused by the reference.

Sharding: 8 cores = 4 batches x 2 query-row halves. Each core:
  - receives x^T (bf16) for its batch with ITS query half's rows FIRST
    (attention is permutation-invariant over key rows, so reordering
    k rows is free; q rows stay in original order within the half);
  - computes full K^T / V for the batch (2x redundant) + Q^T for its half;
  - flash-style attention with scores transposed [k, q]: the two heads of
    a pair run as concurrent 64-contraction PE row tiles; the softmax
    denominator comes free from a fused 128-wide [V_A | ones | V_B]
    stationary weight (scheme "fused": windows [0:128] / [64:192] put
    numerator+denominator of each head in one matmul stream);
  - fc projection on its disjoint 1024 output rows.
Output is a pure concatenation — no collectives, no host reduction.
"""

import math
import sys
from contextlib import ExitStack

import numpy as np
import ml_dtypes

for _p in ("/opt/trn_rl_repo", "/root/.axon_site/_ro/trn_rl_repo"):
    if _p not in sys.path:
        sys.path.insert(0, _p)

import concourse.bass as bass  # noqa: E402
import concourse.tile as tile  # noqa: E402
from concourse import bacc, mybir  # noqa: E402
from concourse.bass_utils import run_bass_kernel_spmd  # noqa: E402

bf16 = ml_dtypes.bfloat16
FP32 = mybir.dt.float32
BF16 = mybir.dt.bfloat16
AF = mybir.ActivationFunctionType

B, S, D, H = 4, 2048, 1024, 16
HD = D // H
SCALE = math.sqrt(HD)


class Cfg:
    def __init__(self, R=2048, Q=1024, Hn=16, D=1024, repeats=1,
                 attn_scheme="coltile", interleave=True, no_ones=False,
                 norm_mode="direct", abufs=3, dbufs=1, fake_exp=False):
        self.fake_exp = fake_exp
        assert R % 128 == 0 and Q % 128 == 0 and Hn % 4 == 0
        self.R, self.Q, self.Hn, self.D = R, Q, Hn, D
        self.FT = D // 128          # feature tiles (proj contraction)
        self.NCT = Hn // 2          # coltiles for Q (and K) = heads/2
        self.NRT = R // 128         # k row tiles
        self.NG = Hn // 4           # head groups of 4
        self.NJ = Hn * 64 // 128    # d-tiles for fc contraction
        self.NQT = Q // 128
        self.NRC = max(1, R // 1024)
        self.repeats = repeats
        self.attn_scheme = attn_scheme
        self.interleave = interleave
        self.no_ones = no_ones
        self.norm_mode = norm_mode
        self.abufs = abufs
        self.dbufs = dbufs
        self.scale = 1.0 / math.sqrt(64.0)


def build_nc(cfg: Cfg, num_devices=8):
    R, Q, Hn, Dm, FT = cfg.R, cfg.Q, cfg.Hn, cfg.D, cfg.FT
    nc = bacc.Bacc("TRN2", target_bir_lowering=False, debug=False,
                   enable_asserts=False, num_devices=num_devices)
    xt_d = nc.dram_tensor("xt", [Dm, R], BF16, kind="ExternalInput").ap()
    wqk_d = nc.dram_tensor("wqk", [Hn, 128, FT, 128], BF16,
                           kind="ExternalInput").ap()
    wv_d = nc.dram_tensor("wv", [FT, 128, Hn * 64], BF16,
                          kind="ExternalInput").ap()
    wfc_d = nc.dram_tensor("wfc", [cfg.NJ, 128, Dm], BF16,
                           kind="ExternalInput").ap()
    bfc_d = nc.dram_tensor("bfc", [128, Dm], FP32, kind="ExternalInput").ap()
    y_d = nc.dram_tensor("y", [Q, Dm], FP32, kind="ExternalOutput").ap()
    with tile.TileContext(nc) as tc:
        with ExitStack() as ctx:
            build_body(ctx, tc, cfg, xt_d, wqk_d, wv_d, wfc_d, bfc_d, y_d)
    nc.finalize()
    return nc


def build_body(ctx, tc, cfg: Cfg, xt_d, wqk_d, wv_d, wfc_d, bfc_d, y_d):
    nc = tc.nc
    R, Q, Hn, Dm, FT = cfg.R, cfg.Q, cfg.Hn, cfg.D, cfg.FT
    NCT, NRT, NG, NJ, NQT, NRC = (cfg.NCT, cfg.NRT, cfg.NG, cfg.NJ,
                                  cfg.NQT, cfg.NRC)
    m65 = cfg.attn_scheme == "m65"

    persist = ctx.enter_context(tc.tile_pool(name="persist", bufs=1))
    wqk_pool = ctx.enter_context(tc.tile_pool(name="wqk", bufs=6))
    wv_pool = ctx.enter_context(tc.tile_pool(name="wv", bufs=2))
    attn_pool = ctx.enter_context(tc.tile_pool(name="attn", bufs=cfg.abufs))
    ysb_pool = ctx.enter_context(tc.tile_pool(name="ysb", bufs=2))
    den_pool = ctx.enter_context(tc.tile_pool(name="den", bufs=cfg.dbufs))
    spool = ctx.enter_context(tc.tile_pool(name="ps_s", bufs=2, space="PSUM"))
    opool = ctx.enter_context(tc.tile_pool(name="ps_o", bufs=2, space="PSUM"))

    for _rep in range(cfg.repeats):
        fused = cfg.attn_scheme in ("fused", "fused3")
        xt_sb = persist.tile([128, FT, R], BF16, tag="xt")
        KT_sb = persist.tile([128, NCT, R], BF16, tag="kt")
        QT_sb = persist.tile([128, NCT, Q], BF16, tag="qt")
        if fused:
            # per head-pair hp: cols 0:64 = V_A, 64:128 = ones, 128:192 = V_B.
            # weight window A = [:, 0:128] -> out = [num_A; den_A],
            # weight window B = [:, 64:192] -> out = [den_B; num_B].
            V_sb = persist.tile([128, NRT, NCT, 192], BF16, tag="v")
            nc.vector.memset(V_sb[:, :, :, 64:128], 1.0)
        else:
            VW = 65 if m65 else 64
            V_sb = persist.tile([128, NRT, Hn, VW], BF16, tag="v")
        OT_sb = persist.tile([128, NJ, Q], BF16, tag="ot")
        wfc_sb = persist.tile([128, NJ, Dm], BF16, tag="wfc")
        bfc_sb = persist.tile([128, Dm], FP32, tag="bfc")
        if m65:
            ones1_sb = persist.tile([1, 64], BF16, tag="ones1")
            nc.vector.memset(ones1_sb[:], 1.0)
            nc.vector.memset(V_sb[:, :, :, 64:65], 1.0)
        elif not fused:
            ones_sb = persist.tile([128, 64], BF16, tag="ones")
            nc.vector.memset(ones_sb[:], 1.0)

        wqk_tiles = {}

        def load_wqk(j):
            t = wqk_pool.tile([128, FT, 128], BF16, tag="wqk", name="wqk_t")
            nc.sync.dma_start(t[:], wqk_d[j])
            wqk_tiles[j] = t

        for ft in range(FT):
            nc.sync.dma_start(xt_sb[:, ft, :], xt_d[ft * 128:(ft + 1) * 128, :])

        def emit_qkv_units(g, split=False):
            """Return closures emitting group g's QKV projection psum-chains.
            split=False: one flat list (wqk+wv loaded by first unit).
            split=True: dict {load_wv, qk, v} for fine-grained scheduling."""
            cts = [2 * g, 2 * g + 1]
            units = []

            state = {}

            def load_wv(g=g):
                if "wv" not in state:
                    wv_sb = wv_pool.tile([128, FT, 256], BF16, tag="wv",
                                         name="wv_t")
                    nc.sync.dma_start(
                        wv_sb[:],
                        wv_d[:, :, g * 256:(g + 1) * 256].rearrange(
                            "f p c -> p f c"))
                    state["wv"] = wv_sb

            def ensure_wqk():
                if "qk" not in state:
                    for ct in cts:
                        load_wqk(ct)
                        load_wqk(NCT + ct)
                    state["qk"] = True

            def ensure_w():
                load_wv()
                ensure_wqk()

            def q_unit(ct):
                def emit():
                    ensure_wqk()
                    ps = spool.tile([128, 1024], FP32, tag="ps_s",
                                    name="ps_q")[:, :Q]
                    for sc in range(0, Q, 512):
                        sn = min(512, Q - sc)
                        for ft in range(FT):
                            nc.tensor.matmul(
                                ps[:, sc:sc + sn], wqk_tiles[ct][:, ft, :],
                                xt_sb[:, ft, sc:sc + sn],
                                start=(ft == 0), stop=(ft == FT - 1))
                    nc.vector.tensor_copy(QT_sb[:, ct, :], ps)
                return emit

            def k_unit(ct, rc):
                def emit():
                    ensure_wqk()
                    rn = min(1024, R - rc * 1024)
                    ps = spool.tile([128, 1024], FP32, tag="ps_s",
                                    name="ps_k")[:, :rn]
                    for sc in range(0, rn, 512):
                        sn = min(512, rn - sc)
                        for ft in range(FT):
                            nc.tensor.matmul(
                                ps[:, sc:sc + sn],
                                wqk_tiles[NCT + ct][:, ft, :],
                                xt_sb[:, ft, rc * 1024 + sc:rc * 1024 + sc + sn],
                                start=(ft == 0), stop=(ft == FT - 1))
                    nc.vector.tensor_copy(
                        KT_sb[:, ct, rc * 1024:rc * 1024 + rn], ps)
                return emit

            def v_unit(rt):
                def emit():
                    load_wv()
                    ps = spool.tile([128, 1024], FP32, tag="ps_s",
                                    name="ps_v")[:, :256]
                    for ft in range(FT):
                        nc.tensor.matmul(
                            ps, xt_sb[:, ft, rt * 128:(rt + 1) * 128],
                            state["wv"][:, ft, :],
                            start=(ft == 0), stop=(ft == FT - 1))
                    if cfg.attn_scheme in ("fused", "fused3"):
                        psr = ps.rearrange("p (h c) -> p h c", c=64)
                        for h in range(4):
                            nc.vector.tensor_copy(
                                V_sb[:, rt, 2 * g + h // 2,
                                     (h % 2) * 128:(h % 2) * 128 + 64],
                                psr[:, h, :])
                    else:
                        nc.vector.tensor_copy(
                            V_sb[:, rt, 4 * g:4 * g + 4, 0:64],
                            ps.rearrange("p (h c) -> p h c", c=64))
                return emit

            qk_units = []
            for ct in cts:
                qk_units.append(q_unit(ct))
                qk_units.append(k_unit(ct, 0))
                if NRC > 1:
                    qk_units.append(k_unit(ct, 1))
            v_units = [v_unit(rt) for rt in range(NRT)]
            if split:
                return {"load_wv": load_wv, "qk": qk_units, "v": v_units}
            return qk_units + v_units

        def norm_and_store(o_, hp, half):
            """divide numerator rows by the fused denominator, write OT."""
            if m65:
                # copy psum out immediately to free the accumulator slot
                st = den_pool.tile([65, 1024], FP32, tag="st",
                                   name="st")[:, :Q]
                nc.vector.tensor_copy(st, o_[0:65, :])
                rcp = den_pool.tile([1, 1024], FP32, tag="rcp",
                                    name="rcp")[:, :Q]
                nc.vector.reciprocal(rcp, st[64:65, :])
                # bf16 hi/lo split so the bf16 broadcast matmul is exact
                hi = den_pool.tile([1, 1024], BF16, tag="rhi",
                                   name="rhi")[:, :Q]
                lo = den_pool.tile([1, 1024], BF16, tag="rlo",
                                   name="rlo")[:, :Q]
                tmp = den_pool.tile([1, 1024], FP32, tag="rtmp",
                                    name="rtmp")[:, :Q]
                nc.vector.tensor_copy(hi, rcp)
                nc.vector.tensor_tensor(tmp, rcp, hi,
                                        mybir.AluOpType.subtract)
                nc.vector.tensor_copy(lo, tmp)
                bc = spool.tile([128, 1024], FP32, tag="ps_s",
                                name="bc")[0:64, :Q]
                for sc in range(0, Q, 512):
                    sn = min(512, Q - sc)
                    s_ = slice(sc, sc + sn)
                    nc.tensor.matmul(bc[:, s_], ones1_sb[:], hi[:, s_],
                                     start=True, stop=False)
                    nc.tensor.matmul(bc[:, s_], ones1_sb[:], lo[:, s_],
                                     start=False, stop=True)
                nc.vector.tensor_mul(OT_sb[half * 64:half * 64 + 64, hp, :],
                                     st[0:64, :], bc)
            elif cfg.no_ones:
                nc.vector.tensor_copy(OT_sb[half * 64:half * 64 + 64, hp, :],
                                      o_[0:64, :])
            elif cfg.norm_mode == "copyout":
                st = den_pool.tile([128, 1024], FP32, tag="stc",
                                   name="stc")[:, :Q]
                nc.vector.tensor_copy(st, o_[:, :])
                den = den_pool.tile([64, 1024], FP32, tag="den",
                                    name="den")[:, :Q]
                nc.vector.reciprocal(den, st[64:128, :])
                nc.vector.tensor_mul(OT_sb[half * 64:half * 64 + 64, hp, :],
                                     st[0:64, :], den)
            else:
                den = den_pool.tile([64, 1024], FP32, tag="den",
                                    name="den")[:, :Q]
                nc.vector.reciprocal(den, o_[64:128, :])
                nc.vector.tensor_mul(OT_sb[half * 64:half * 64 + 64, hp, :],
                                     o_[0:64, :], den)

        def norm_pair_fused3(oA, oB, hp):
            """Pair norm ordered for early psum release: oA freed by one
            fast bf16 copy; oB normalized straight from psum (legal: the
            two-SBUF-input base-partition rule doesn't bind PSUM reads)."""
            st = den_pool.tile([128, 1024], BF16, tag="stb",
                               name="stb")[:, :Q]
            nc.vector.tensor_copy(st, oA[:, :])
            denB = den_pool.tile([64, 1024], FP32, tag="den",
                                 name="den")[:, :Q]
            nc.vector.reciprocal(denB, oB[0:64, :])
            nc.vector.tensor_mul(OT_sb[64:128, hp, :], oB[64:128, :], denB)
            denA = den_pool.tile([64, 1024], BF16, tag="denb",
                                 name="denb")[:, :Q]
            with nc.allow_low_precision(
                    reason="softmax num/den rounded to bf16; OT is bf16"):
                nc.vector.reciprocal(denA, st[64:128, :])
                nc.vector.tensor_mul(OT_sb[0:64, hp, :], st[0:64, :], denA)

        def attn_step(hp, kt, oA, oB):
            psA = spool.tile([128, 1024], FP32, tag="ps_s", name="psA")[:, :Q]
            psB = spool.tile([128, 1024], FP32, tag="ps_s", name="psB")[:, :Q]
            for sc in range(0, Q, 512):
                sn = min(512, Q - sc)
                nc.tensor.matmul(
                    psA[:, sc:sc + sn],
                    KT_sb[0:64, hp, kt * 128:(kt + 1) * 128],
                    QT_sb[0:64, hp, sc:sc + sn], start=True, stop=True)
                nc.tensor.matmul(
                    psB[:, sc:sc + sn],
                    KT_sb[64:128, hp, kt * 128:(kt + 1) * 128],
                    QT_sb[64:128, hp, sc:sc + sn], start=True, stop=True)
            aA = attn_pool.tile([128, Q], BF16, tag="aT", name="aA")
            aB = attn_pool.tile([128, Q], BF16, tag="aT", name="aB")
            if cfg.fake_exp:
                # diagnostic only: replaces ACT exp with a DVE copy to
                # isolate the ACT chain's contribution to the critical path
                nc.vector.tensor_copy(aA[:], psA)
                nc.vector.tensor_copy(aB[:], psB)
            else:
                nc.scalar.activation(aA[:], psA, AF.Exp, scale=cfg.scale)
                nc.scalar.activation(aB[:], psB, AF.Exp, scale=cfg.scale)
            st, sp = (kt == 0), (kt == NRT - 1)
            for sc in range(0, Q, 512):
                sn = min(512, Q - sc)
                s_ = slice(sc, sc + sn)
                if cfg.attn_scheme in ("fused", "fused3"):
                    nc.tensor.matmul(oA[:, s_], V_sb[:, kt, hp, 0:128],
                                     aA[:, s_], start=st, stop=sp)
                    nc.tensor.matmul(oB[:, s_], V_sb[:, kt, hp, 64:192],
                                     aB[:, s_], start=st, stop=sp)
                elif m65:
                    nc.tensor.matmul(oA[0:65, s_], V_sb[:, kt, 2 * hp, :],
                                     aA[:, s_], start=st, stop=sp)
                    nc.tensor.matmul(oB[0:65, s_], V_sb[:, kt, 2 * hp + 1, :],
                                     aB[:, s_], start=st, stop=sp)
                elif cfg.attn_scheme == "swap":
                    # complementary col-groups across tiles: V_A(grp0,oA) ||
                    # V_B(grp64,oB), then ones_A(grp64,oA) || ones_B(grp0,oB).
                    # Per-tile writer order identical to the safe layout.
                    nc.tensor.matmul(oA[0:64, s_],
                                     V_sb[:, kt, 2 * hp, 0:64],
                                     aA[:, s_], start=st, stop=sp)
                    nc.tensor.matmul(oB[64:128, s_],
                                     V_sb[:, kt, 2 * hp + 1, 0:64],
                                     aB[:, s_], start=st, stop=sp)
                    nc.tensor.matmul(oA[64:128, s_], ones_sb[:],
                                     aA[:, s_], start=st, stop=sp)
                    nc.tensor.matmul(oB[0:64, s_], ones_sb[:],
                                     aB[:, s_], start=st, stop=sp)
                else:
                    nc.tensor.matmul(oA[0:64, s_],
                                     V_sb[:, kt, 2 * hp, 0:64],
                                     aA[:, s_], start=st, stop=sp)
                    if not cfg.no_ones:
                        nc.tensor.matmul(oA[64:128, s_], ones_sb[:],
                                         aA[:, s_], start=st, stop=sp)
                    nc.tensor.matmul(oB[0:64, s_],
                                     V_sb[:, kt, 2 * hp + 1, 0:64],
                                     aB[:, s_], start=st, stop=sp)
                    if not cfg.no_ones:
                        nc.tensor.matmul(oB[64:128, s_], ones_sb[:],
                                         aB[:, s_], start=st, stop=sp)

        # ---- main loop: group attention with next group's QKV interleaved
        if cfg.attn_scheme == "fused3":
            # v3 schedule: only Q/K of group 0 run up-front; V units drip
            # just-in-time into their own group's pair-0 attention steps
            # (unit rt before the kt=rt attn-out); next group's Q/K units
            # drip evenly across all 32 steps.
            u0 = emit_qkv_units(0, split=True)
            u0["load_wv"]()
            for u in u0["qk"]:
                u()
            nc.sync.dma_start(wfc_sb[:], wfc_d.rearrange("j p d -> p j d"))
            nc.sync.dma_start(bfc_sb[:], bfc_d[:])
            v_cur = u0["v"]
            for g in range(NG):
                nxt = emit_qkv_units(g + 1, split=True) if g + 1 < NG else None
                if nxt is not None:
                    nxt["load_wv"]()
                next_qk = nxt["qk"] if nxt is not None else []
                steps = [(hp, kt) for hp in (2 * g, 2 * g + 1)
                         for kt in range(NRT)]
                o_tiles = {}
                ui = 0
                for si, (hp, kt) in enumerate(steps):
                    if si < len(v_cur):
                        v_cur[si]()
                    if kt == 0:
                        o_tiles[hp] = (
                            opool.tile([128, 1024], FP32, tag="ps_o",
                                       name="oA")[:, :Q],
                            opool.tile([128, 1024], FP32, tag="ps_o",
                                       name="oB")[:, :Q])
                    oA, oB = o_tiles[hp]
                    attn_step(hp, kt, oA, oB)
                    want = (si + 1) * len(next_qk) // len(steps)
                    while ui < want:
                        next_qk[ui]()
                        ui += 1
                    if kt == NRT - 1:
                        norm_pair_fused3(oA, oB, hp)
                v_cur = nxt["v"] if nxt is not None else []
        if cfg.attn_scheme != "fused3":
            units = emit_qkv_units(0)
            for u in units:
                u()
            # fc weights aren't needed until the tail — keep them off the
            # critical-path DMA window at kernel start
            nc.sync.dma_start(wfc_sb[:], wfc_d.rearrange("j p d -> p j d"))
            nc.sync.dma_start(bfc_sb[:], bfc_d[:])
        for g in range(NG if cfg.attn_scheme != "fused3" else 0):
            next_units = emit_qkv_units(g + 1) if g + 1 < NG else []
            steps = [(hp, kt) for hp in (2 * g, 2 * g + 1)
                     for kt in range(NRT)]
            o_tiles = {}
            ui = 0
            for si, (hp, kt) in enumerate(steps):
                if kt == 0:
                    o_tiles[hp] = (
                        opool.tile([128, 1024], FP32, tag="ps_o",
                                   name="oA")[:, :Q],
                        opool.tile([128, 1024], FP32, tag="ps_o",
                                   name="oB")[:, :Q])
                oA, oB = o_tiles[hp]
                attn_step(hp, kt, oA, oB)
                if cfg.interleave:
                    want = (si + 1) * len(next_units) // len(steps)
                    while ui < want:
                        next_units[ui]()
                        ui += 1
                if kt == NRT - 1:
                    norm_and_store(oA, hp, 0)
                    if cfg.attn_scheme in ("swap", "fused"):
                        den = den_pool.tile([64, 1024], FP32, tag="den",
                                            name="denS")[:, :Q]
                        nc.vector.reciprocal(den, oB[0:64, :])
                        nc.vector.tensor_mul(OT_sb[64:128, hp, :],
                                             oB[64:128, :], den)
                    else:
                        norm_and_store(oB, hp, 1)
            while ui < len(next_units):
                next_units[ui]()
                ui += 1
            if not cfg.interleave:
                pass

        # ---- fc ----
        for qt in range(NQT):
            ps = spool.tile([128, 1024], FP32, tag="ps_s",
                            name="ps_fc")[:, :Dm]
            for cc in range(0, Dm, 512):
                for j in range(NJ):
                    nc.tensor.matmul(
                        ps[:, cc:cc + 512],
                        OT_sb[:, j, qt * 128:(qt + 1) * 128],
                        wfc_sb[:, j, cc:cc + 512],
                        start=(j == 0), stop=(j == NJ - 1))
            yt = ysb_pool.tile([128, Dm], FP32, tag="y", name="yt")
            nc.vector.tensor_add(yt[:], ps, bfc_sb[:])
            nc.sync.dma_start(y_d[qt * 128:(qt + 1) * 128, :], yt[:])


# ---------------- host side ----------------

def prep_core_inputs(cfg: Cfg, xb_perm, W_qkv, W_fc, b_fc):
    """xb_perm: [R, D] f32, rows already permuted (this core's q rows first)."""
    Dm, Hn, FT, NCT, NJ = cfg.D, cfg.Hn, cfg.FT, cfg.NCT, cfg.NJ
    xt = np.ascontiguousarray(xb_perm.T).astype(bf16)
    Wq = W_qkv[:, :NCT * 128]
    Wk = W_qkv[:, Dm:Dm + NCT * 128]
    Wv = W_qkv[:, 2 * Dm:2 * Dm + Hn * 64]
    wq_t = Wq.reshape(FT, 128, NCT, 128).transpose(2, 1, 0, 3)
    wk_t = Wk.reshape(FT, 128, NCT, 128).transpose(2, 1, 0, 3)
    wqk = np.ascontiguousarray(
        np.concatenate([wq_t, wk_t], axis=0)).astype(bf16)
    wv = np.ascontiguousarray(Wv.reshape(FT, 128, Hn * 64)).astype(bf16)
    wfc = np.ascontiguousarray(
        W_fc[:NJ * 128].reshape(NJ, 128, Dm)).astype(bf16)
    bfc = np.ascontiguousarray(
        np.broadcast_to(b_fc.astype(np.float32), (128, Dm)))
    return {"xt": xt, "wqk": wqk, "wv": wv, "wfc": wfc, "bfc": bfc}


_CACHE = {}

SCHEME = "fused+ab4"


def _get_nc(repeats=1):
    key = ("nc", repeats, SCHEME)
    if key not in _CACHE:
        scheme = SCHEME
        fake_exp = scheme.endswith("+fakeexp")
        if fake_exp:
            scheme = scheme[:-len("+fakeexp")]
        abufs = 3
        if "+ab" in scheme:
            scheme, ab = scheme.split("+ab")
            abufs = int(ab)
        nm = "bf16out" if scheme == "fused3" else "direct"
        _CACHE[key] = build_nc(Cfg(R=S, Q=S // 2, Hn=H, D=D, repeats=repeats,
                                   attn_scheme=scheme, norm_mode=nm,
                                   abufs=abufs, fake_exp=fake_exp))
    return _CACHE[key]


def make_in_maps(x, W_qkv, W_fc, b_fc):
    cfg = Cfg(R=S, Q=S // 2, Hn=H, D=D)
    x = np.asarray(x, dtype=np.float32)
    in_maps = []
    for c in range(8):
        b, half = divmod(c, 2)
        r0 = half * (S // 2)
        order = np.concatenate([
            np.arange(r0, r0 + S // 2),
            np.arange(0, r0),
            np.arange(r0 + S // 2, S),
        ])
        xb = x[b][order]
        in_maps.append(prep_core_inputs(
            cfg, xb, np.asarray(W_qkv, np.float32),
            np.asarray(W_fc, np.float32), np.asarray(b_fc, np.float32)))
    return in_maps


def kernel(x, char_ids, seq_len, mask, W_qkv, W_fc, b_fc):
    """Full inputs in, full [B, S, D] float32 output out."""
    import os
    # the axon NTFF trace hook is unavailable in this container; make sure
    # an inherited BASS_TRACE=1 cannot send us down that (crashing) path
    os.environ["BASS_NEVER_TRACE"] = "1"
    nc = _get_nc(repeats=1)
    in_maps = make_in_maps(x, W_qkv, W_fc, b_fc)
    res = run_bass_kernel_spmd(nc, in_maps, core_ids=list(range(8)))
    out = np.empty((B, S, D), dtype=np.float32)
    for c in range(8):
        b, half = divmod(c, 2)
        r0 = half * (S // 2)
        out[b, r0:r0 + S // 2, :] = res.results[c]["y"]
    return out



# revision 32
# speedup vs baseline: 29.4831x; 1.1705x over previous
"""Self-contained TRN2 Bass kernel for nn_MultiHeadAttn_91010357002583.

Multi-head attention (B=4, S=2048, D=1024, H=16, hd=64), eval mode,
mask all-ones, char_ids/seq_len unused by the reference.

Sharding: 8 cores = 4 batches x 2 query-row halves. Each core:
  - receives x^T (bf16) for its batch with ITS query half's rows FIRST
    (attention is permutation-invariant over key rows, so reordering
    k rows is free; q rows stay in original order within the half);
  - computes full K^T / V for the batch (2x redundant) + Q^T for its half;
  - flash-style attention with scores transposed [k, q]: the two heads of
    a pair run as concurrent 64-contraction PE row tiles; the softmax
    denominator comes free from a fused 128-wide [V_A | ones | V_B]
    stationary weight (scheme "fused": windows [0:128] / [64:192] put
    numerator+denominator of each head in one matmul stream);
  - fc projection on its disjoint 1024 output rows.
Output is a pure concatenation — no collectives, no host reduction.
"""

import math
import sys
from contextlib import ExitStack

import numpy as np
import ml_dtypes

for _p in ("/opt/trn_rl_repo", "/root/.axon_site/_ro/trn_rl_repo"):
    if _p not in sys.path:
        sys.path.insert(0, _p)

import concourse.bass as bass  # noqa: E402
import concourse.tile as tile  # noqa: E402
from concourse import bacc, mybir  # noqa: E402
from concourse.bass_utils import run_bass_kernel_spmd  # noqa: E402

bf16 = ml_dtypes.bfloat16
FP32 = mybir.dt.float32
BF16 = mybir.dt.bfloat16
AF = mybir.ActivationFunctionType

B, S, D, H = 4, 2048, 1024, 16
HD = D // H
SCALE = math.sqrt(HD)


class Cfg:
    def __init__(self, R=2048, Q=1024, Hn=16, D=1024, repeats=1,
                 attn_scheme="coltile", interleave=True, no_ones=False,
                 norm_mode="direct", abufs=3, dbufs=1, fake_exp=False):
        self.fake_exp = fake_exp
        assert R % 128 == 0 and Q % 128 == 0 and Hn % 4 == 0
        self.R, self.Q, self.Hn, self.D = R, Q, Hn, D
        self.FT = D // 128          # feature tiles (proj contraction)
        self.NCT = Hn // 2          # coltiles for Q (and K) = heads/2
        self.NRT = R // 128         # k row tiles
        self.NG = Hn // 4           # head groups of 4
        self.NJ = Hn * 64 // 128    # d-tiles for fc contraction
        self.NQT = Q // 128
        self.NRC = max(1, R // 1024)
        self.repeats = repeats
        self.attn_scheme = attn_scheme
        self.interleave = interleave
        self.no_ones = no_ones
        self.norm_mode = norm_mode
        self.abufs = abufs
        self.dbufs = dbufs
        self.scale = 1.0 / math.sqrt(64.0)


def build_nc(cfg: Cfg, num_devices=8):
    R, Q, Hn, Dm, FT = cfg.R, cfg.Q, cfg.Hn, cfg.D, cfg.FT
    nc = bacc.Bacc("TRN2", target_bir_lowering=False, debug=False,
                   enable_asserts=False, num_devices=num_devices)
    xt_d = nc.dram_tensor("xt", [Dm, R], BF16, kind="ExternalInput").ap()
    wqk_d = nc.dram_tensor("wqk", [Hn, 128, FT, 128], BF16,
                           kind="ExternalInput").ap()
    wv_d = nc.dram_tensor("wv", [FT, 128, Hn * 64], BF16,
                          kind="ExternalInput").ap()
    wfc_d = nc.dram_tensor("wfc", [cfg.NJ, 128, Dm], BF16,
                           kind="ExternalInput").ap()
    bfc_d = nc.dram_tensor("bfc", [128, Dm], FP32, kind="ExternalInput").ap()
    y_d = nc.dram_tensor("y", [Q, Dm], FP32, kind="ExternalOutput").ap()
    with tile.TileContext(nc) as tc:
        with ExitStack() as ctx:
            build_body(ctx, tc, cfg, xt_d, wqk_d, wv_d, wfc_d, bfc_d, y_d)
    nc.finalize()
    return nc


def build_body(ctx, tc, cfg: Cfg, xt_d, wqk_d, wv_d, wfc_d, bfc_d, y_d):
    nc = tc.nc
    R, Q, Hn, Dm, FT = cfg.R, cfg.Q, cfg.Hn, cfg.D, cfg.FT
    NCT, NRT, NG, NJ, NQT, NRC = (cfg.NCT, cfg.NRT, cfg.NG, cfg.NJ,
                                  cfg.NQT, cfg.NRC)
    m65 = cfg.attn_scheme == "m65"

    persist = ctx.enter_context(tc.tile_pool(name="persist", bufs=1))
    wqk_pool = ctx.enter_context(tc.tile_pool(name="wqk", bufs=6))
    wv_pool = ctx.enter_context(tc.tile_pool(name="wv", bufs=2))
    attn_pool = ctx.enter_context(tc.tile_pool(name="attn", bufs=cfg.abufs))
    ysb_pool = ctx.enter_context(tc.tile_pool(name="ysb", bufs=2))
    den_pool = ctx.enter_context(tc.tile_pool(name="den", bufs=cfg.dbufs))
    spool = ctx.enter_context(tc.tile_pool(name="ps_s", bufs=2, space="PSUM"))
    opool = ctx.enter_context(tc.tile_pool(name="ps_o", bufs=2, space="PSUM"))

    for _rep in range(cfg.repeats):
        fused = cfg.attn_scheme in ("fused", "fused3")
        xt_sb = persist.tile([128, FT, R], BF16, tag="xt")
        KT_sb = persist.tile([128, NCT, R], BF16, tag="kt")
        QT_sb = persist.tile([128, NCT, Q], BF16, tag="qt")
        if fused:
            # per head-pair hp: cols 0:64 = V_A, 64:128 = ones, 128:192 = V_B.
            # weight window A = [:, 0:128] -> out = [num_A; den_A],
            # weight window B = [:, 64:192] -> out = [den_B; num_B].
            V_sb = persist.tile([128, NRT, NCT, 192], BF16, tag="v")
            nc.vector.memset(V_sb[:, :, :, 64:128], 1.0)
        else:
            VW = 65 if m65 else 64
            V_sb = persist.tile([128, NRT, Hn, VW], BF16, tag="v")
        OT_sb = persist.tile([128, NJ, Q], BF16, tag="ot")
        wfc_sb = persist.tile([128, NJ, Dm], BF16, tag="wfc")
        bfc_sb = persist.tile([128, Dm], FP32, tag="bfc")
        if m65:
            ones1_sb = persist.tile([1, 64], BF16, tag="ones1")
            nc.vector.memset(ones1_sb[:], 1.0)
            nc.vector.memset(V_sb[:, :, :, 64:65], 1.0)
        elif not fused:
            ones_sb = persist.tile([128, 64], BF16, tag="ones")
            nc.vector.memset(ones_sb[:], 1.0)

        wqk_tiles = {}

        def load_wqk(j):
            t = wqk_pool.tile([128, FT, 128], BF16, tag="wqk", name="wqk_t")
            nc.sync.dma_start(t[:], wqk_d[j])
            wqk_tiles[j] = t

        for ft in range(FT):
            nc.sync.dma_start(xt_sb[:, ft, :], xt_d[ft * 128:(ft + 1) * 128, :])

        def emit_qkv_units(g, split=False):
            """Return closures emitting group g's QKV projection psum-chains.
            split=False: one flat list (wqk+wv loaded by first unit).
            split=True: dict {load_wv, qk, v} for fine-grained scheduling."""
            cts = [2 * g, 2 * g + 1]
            units = []

            state = {}

            def load_wv(g=g):
                if "wv" not in state:
                    wv_sb = wv_pool.tile([128, FT, 256], BF16, tag="wv",
                                         name="wv_t")
                    nc.sync.dma_start(
                        wv_sb[:],
                        wv_d[:, :, g * 256:(g + 1) * 256].rearrange(
                            "f p c -> p f c"))
                    state["wv"] = wv_sb

            def ensure_wqk():
                if "qk" not in state:
                    for ct in cts:
                        load_wqk(ct)
                        load_wqk(NCT + ct)
                    state["qk"] = True

            def ensure_w():
                load_wv()
                ensure_wqk()

            def q_unit(ct):
                def emit():
                    ensure_wqk()
                    ps = spool.tile([128, 1024], FP32, tag="ps_s",
                                    name="ps_q")[:, :Q]
                    for sc in range(0, Q, 512):
                        sn = min(512, Q - sc)
                        for ft in range(FT):
                            nc.tensor.matmul(
                                ps[:, sc:sc + sn], wqk_tiles[ct][:, ft, :],
                                xt_sb[:, ft, sc:sc + sn],
                                start=(ft == 0), stop=(ft == FT - 1))
                    nc.vector.tensor_copy(QT_sb[:, ct, :], ps)
                return emit

            def k_unit(ct, rc):
                def emit():
                    ensure_wqk()
                    rn = min(1024, R - rc * 1024)
                    ps = spool.tile([128, 1024], FP32, tag="ps_s",
                                    name="ps_k")[:, :rn]
                    for sc in range(0, rn, 512):
                        sn = min(512, rn - sc)
                        for ft in range(FT):
                            nc.tensor.matmul(
                                ps[:, sc:sc + sn],
                                wqk_tiles[NCT + ct][:, ft, :],
                                xt_sb[:, ft, rc * 1024 + sc:rc * 1024 + sc + sn],
                                start=(ft == 0), stop=(ft == FT - 1))
                    nc.vector.tensor_copy(
                        KT_sb[:, ct, rc * 1024:rc * 1024 + rn], ps)
                return emit

            def v_unit(rt):
                def emit():
                    load_wv()
                    ps = spool.tile([128, 1024], FP32, tag="ps_s",
                                    name="ps_v")[:, :256]
                    for ft in range(FT):
                        nc.tensor.matmul(
                            ps, xt_sb[:, ft, rt * 128:(rt + 1) * 128],
                            state["wv"][:, ft, :],
                            start=(ft == 0), stop=(ft == FT - 1))
                    if cfg.attn_scheme in ("fused", "fused3"):
                        psr = ps.rearrange("p (h c) -> p h c", c=64)
                        for h in range(4):
                            nc.vector.tensor_copy(
                                V_sb[:, rt, 2 * g + h // 2,
                                     (h % 2) * 128:(h % 2) * 128 + 64],
                                psr[:, h, :])
                    else:
                        nc.vector.tensor_copy(
                            V_sb[:, rt, 4 * g:4 * g + 4, 0:64],
                            ps.rearrange("p (h c) -> p h c", c=64))
                return emit

            qk_units = []
            for ct in cts:
                qk_units.append(q_unit(ct))
                qk_units.append(k_unit(ct, 0))
                if NRC > 1:
                    qk_units.append(k_unit(ct, 1))
            v_units = [v_unit(rt) for rt in range(NRT)]
            if split:
                return {"load_wv": load_wv, "qk": qk_units, "v": v_units}
            return qk_units + v_units

        def norm_and_store(o_, hp, half):
            """divide numerator rows by the fused denominator, write OT."""
            if m65:
                # copy psum out immediately to free the accumulator slot
                st = den_pool.tile([65, 1024], FP32, tag="st",
                                   name="st")[:, :Q]
                nc.vector.tensor_copy(st, o_[0:65, :])
                rcp = den_pool.tile([1, 1024], FP32, tag="rcp",
                                    name="rcp")[:, :Q]
                nc.vector.reciprocal(rcp, st[64:65, :])
                # bf16 hi/lo split so the bf16 broadcast matmul is exact
                hi = den_pool.tile([1, 1024], BF16, tag="rhi",
                                   name="rhi")[:, :Q]
                lo = den_pool.tile([1, 1024], BF16, tag="rlo",
                                   name="rlo")[:, :Q]
                tmp = den_pool.tile([1, 1024], FP32, tag="rtmp",
                                    name="rtmp")[:, :Q]
                nc.vector.tensor_copy(hi, rcp)
                nc.vector.tensor_tensor(tmp, rcp, hi,
                                        mybir.AluOpType.subtract)
                nc.vector.tensor_copy(lo, tmp)
                bc = spool.tile([128, 1024], FP32, tag="ps_s",
                                name="bc")[0:64, :Q]
                for sc in range(0, Q, 512):
                    sn = min(512, Q - sc)
                    s_ = slice(sc, sc + sn)
                    nc.tensor.matmul(bc[:, s_], ones1_sb[:], hi[:, s_],
                                     start=True, stop=False)
                    nc.tensor.matmul(bc[:, s_], ones1_sb[:], lo[:, s_],
                                     start=False, stop=True)
                nc.vector.tensor_mul(OT_sb[half * 64:half * 64 + 64, hp, :],
                                     st[0:64, :], bc)
            elif cfg.no_ones:
                nc.vector.tensor_copy(OT_sb[half * 64:half * 64 + 64, hp, :],
                                      o_[0:64, :])
            elif cfg.norm_mode == "copyout":
                st = den_pool.tile([128, 1024], FP32, tag="stc",
                                   name="stc")[:, :Q]
                nc.vector.tensor_copy(st, o_[:, :])
                den = den_pool.tile([64, 1024], FP32, tag="den",
                                    name="den")[:, :Q]
                nc.vector.reciprocal(den, st[64:128, :])
                nc.vector.tensor_mul(OT_sb[half * 64:half * 64 + 64, hp, :],
                                     st[0:64, :], den)
            else:
                den = den_pool.tile([64, 1024], FP32, tag="den",
                                    name="den")[:, :Q]
                nc.vector.reciprocal(den, o_[64:128, :])
                nc.vector.tensor_mul(OT_sb[half * 64:half * 64 + 64, hp, :],
                                     o_[0:64, :], den)

        def norm_pair_fused3(oA, oB, hp):
            """Pair norm ordered for early psum release: oA freed by one
            fast bf16 copy; oB normalized straight from psum (legal: the
            two-SBUF-input base-partition rule doesn't bind PSUM reads)."""
            st = den_pool.tile([128, 1024], BF16, tag="stb",
                               name="stb")[:, :Q]
            nc.vector.tensor_copy(st, oA[:, :])
            denB = den_pool.tile([64, 1024], BF16, tag="denb2",
                                 name="denb2")[:, :Q]
            with nc.allow_low_precision(
                    reason="softmax den rounded to bf16; OT is bf16"):
                nc.vector.reciprocal(denB, oB[0:64, :])
                nc.vector.tensor_mul(OT_sb[64:128, hp, :], oB[64:128, :],
                                     denB)
            denA = den_pool.tile([64, 1024], BF16, tag="denb",
                                 name="denb")[:, :Q]
            with nc.allow_low_precision(
                    reason="softmax num/den rounded to bf16; OT is bf16"):
                nc.vector.reciprocal(denA, st[64:128, :])
                nc.vector.tensor_mul(OT_sb[0:64, hp, :], st[0:64, :], denA)

        def attn_step(hp, kt, oA, oB):
            psA = spool.tile([128, 1024], FP32, tag="ps_s", name="psA")[:, :Q]
            psB = spool.tile([128, 1024], FP32, tag="ps_s", name="psB")[:, :Q]
            for sc in range(0, Q, 512):
                sn = min(512, Q - sc)
                nc.tensor.matmul(
                    psA[:, sc:sc + sn],
                    KT_sb[0:64, hp, kt * 128:(kt + 1) * 128],
                    QT_sb[0:64, hp, sc:sc + sn], start=True, stop=True)
                nc.tensor.matmul(
                    psB[:, sc:sc + sn],
                    KT_sb[64:128, hp, kt * 128:(kt + 1) * 128],
                    QT_sb[64:128, hp, sc:sc + sn], start=True, stop=True)
            aA = attn_pool.tile([128, Q], BF16, tag="aT", name="aA")
            aB = attn_pool.tile([128, Q], BF16, tag="aT", name="aB")
            if cfg.fake_exp:
                # diagnostic only: replaces ACT exp with a DVE copy to
                # isolate the ACT chain's contribution to the critical path
                nc.vector.tensor_copy(aA[:], psA)
                nc.vector.tensor_copy(aB[:], psB)
            else:
                nc.scalar.activation(aA[:], psA, AF.Exp, scale=cfg.scale)
                nc.scalar.activation(aB[:], psB, AF.Exp, scale=cfg.scale)
            st, sp = (kt == 0), (kt == NRT - 1)
            for sc in range(0, Q, 512):
                sn = min(512, Q - sc)
                s_ = slice(sc, sc + sn)
                if cfg.attn_scheme in ("fused", "fused3"):
                    nc.tensor.matmul(oA[:, s_], V_sb[:, kt, hp, 0:128],
                                     aA[:, s_], start=st, stop=sp)
                    nc.tensor.matmul(oB[:, s_], V_sb[:, kt, hp, 64:192],
                                     aB[:, s_], start=st, stop=sp)
                elif m65:
                    nc.tensor.matmul(oA[0:65, s_], V_sb[:, kt, 2 * hp, :],
                                     aA[:, s_], start=st, stop=sp)
                    nc.tensor.matmul(oB[0:65, s_], V_sb[:, kt, 2 * hp + 1, :],
                                     aB[:, s_], start=st, stop=sp)
                elif cfg.attn_scheme == "swap":
                    # complementary col-groups across tiles: V_A(grp0,oA) ||
                    # V_B(grp64,oB), then ones_A(grp64,oA) || ones_B(grp0,oB).
                    # Per-tile writer order identical to the safe layout.
                    nc.tensor.matmul(oA[0:64, s_],
                                     V_sb[:, kt, 2 * hp, 0:64],
                                     aA[:, s_], start=st, stop=sp)
                    nc.tensor.matmul(oB[64:128, s_],
                                     V_sb[:, kt, 2 * hp + 1, 0:64],
                                     aB[:, s_], start=st, stop=sp)
                    nc.tensor.matmul(oA[64:128, s_], ones_sb[:],
                                     aA[:, s_], start=st, stop=sp)
                    nc.tensor.matmul(oB[0:64, s_], ones_sb[:],
                                     aB[:, s_], start=st, stop=sp)
                else:
                    nc.tensor.matmul(oA[0:64, s_],
                                     V_sb[:, kt, 2 * hp, 0:64],
                                     aA[:, s_], start=st, stop=sp)
                    if not cfg.no_ones:
                        nc.tensor.matmul(oA[64:128, s_], ones_sb[:],
                                         aA[:, s_], start=st, stop=sp)
                    nc.tensor.matmul(oB[0:64, s_],
                                     V_sb[:, kt, 2 * hp + 1, 0:64],
                                     aB[:, s_], start=st, stop=sp)
                    if not cfg.no_ones:
                        nc.tensor.matmul(oB[64:128, s_], ones_sb[:],
                                         aB[:, s_], start=st, stop=sp)

        # ---- main loop: group attention with next group's QKV interleaved
        if cfg.attn_scheme == "fused3":
            # v3 schedule: only Q/K of group 0 run up-front; V units drip
            # just-in-time into their own group's pair-0 attention steps
            # (unit rt before the kt=rt attn-out); next group's Q/K units
            # drip evenly across all 32 steps.
            u0 = emit_qkv_units(0, split=True)
            u0["load_wv"]()
            for u in u0["qk"]:
                u()
            nc.sync.dma_start(wfc_sb[:], wfc_d.rearrange("j p d -> p j d"))
            nc.sync.dma_start(bfc_sb[:], bfc_d[:])
            v_cur = u0["v"]
            for g in range(NG):
                nxt = emit_qkv_units(g + 1, split=True) if g + 1 < NG else None
                if nxt is not None:
                    nxt["load_wv"]()
                next_qk = nxt["qk"] if nxt is not None else []
                steps = [(hp, kt) for hp in (2 * g, 2 * g + 1)
                         for kt in range(NRT)]
                o_tiles = {}
                ui = 0
                for si, (hp, kt) in enumerate(steps):
                    if si < len(v_cur):
                        v_cur[si]()
                    if kt == 0:
                        o_tiles[hp] = (
                            opool.tile([128, 1024], FP32, tag="ps_o",
                                       name="oA")[:, :Q],
                            opool.tile([128, 1024], FP32, tag="ps_o",
                                       name="oB")[:, :Q])
                    oA, oB = o_tiles[hp]
                    attn_step(hp, kt, oA, oB)
                    want = (si + 1) * len(next_qk) // len(steps)
                    while ui < want:
                        next_qk[ui]()
                        ui += 1
                    if kt == NRT - 1:
                        norm_pair_fused3(oA, oB, hp)
                v_cur = nxt["v"] if nxt is not None else []
        if cfg.attn_scheme != "fused3":
            units = emit_qkv_units(0)
            for u in units:
                u()
            # fc weights aren't needed until the tail — keep them off the
            # critical-path DMA window at kernel start
            nc.sync.dma_start(wfc_sb[:], wfc_d.rearrange("j p d -> p j d"))
            nc.sync.dma_start(bfc_sb[:], bfc_d[:])
        for g in range(NG if cfg.attn_scheme != "fused3" else 0):
            next_units = emit_qkv_units(g + 1) if g + 1 < NG else []
            steps = [(hp, kt) for hp in (2 * g, 2 * g + 1)
                     for kt in range(NRT)]
            o_tiles = {}
            ui = 0
            for si, (hp, kt) in enumerate(steps):
                if kt == 0:
                    o_tiles[hp] = (
                        opool.tile([128, 1024], FP32, tag="ps_o",
                                   name="oA")[:, :Q],
                        opool.tile([128, 1024], FP32, tag="ps_o",
                                   name="oB")[:, :Q])
                oA, oB = o_tiles[hp]
                attn_step(hp, kt, oA, oB)
                if cfg.interleave:
                    want = (si + 1) * len(next_units) // len(steps)
                    while ui < want:
                        next_units[ui]()
                        ui += 1
                if kt == NRT - 1:
                    if (cfg.attn_scheme == "fused"
                            and cfg.norm_mode == "bf16out"):
                        norm_pair_fused3(oA, oB, hp)
                    else:
                        norm_and_store(oA, hp, 0)
                        if cfg.attn_scheme in ("swap", "fused"):
                            den = den_pool.tile([64, 1024], FP32, tag="den",
                                                name="denS")[:, :Q]
                            nc.vector.reciprocal(den, oB[0:64, :])
                            nc.vector.tensor_mul(OT_sb[64:128, hp, :],
                                                 oB[64:128, :], den)
                        else:
                            norm_and_store(oB, hp, 1)
            while ui < len(next_units):
                next_units[ui]()
                ui += 1
            if not cfg.interleave:
                pass

        # ---- fc ----
        for qt in range(NQT):
            ps = spool.tile([128, 1024], FP32, tag="ps_s",
                            name="ps_fc")[:, :Dm]
            for cc in range(0, Dm, 512):
                for j in range(NJ):
                    nc.tensor.matmul(
                        ps[:, cc:cc + 512],
                        OT_sb[:, j, qt * 128:(qt + 1) * 128],
                        wfc_sb[:, j, cc:cc + 512],
                        start=(j == 0), stop=(j == NJ - 1))
            yt = ysb_pool.tile([128, Dm], FP32, tag="y", name="yt")
            nc.vector.tensor_add(yt[:], ps, bfc_sb[:])
            nc.sync.dma_start(y_d[qt * 128:(qt + 1) * 128, :], yt[:])


# ---------------- host side ----------------

def prep_core_inputs(cfg: Cfg, xb_perm, W_qkv, W_fc, b_fc):
    """xb_perm: [R, D] f32, rows already permuted (this core's q rows first)."""
    Dm, Hn, FT, NCT, NJ = cfg.D, cfg.Hn, cfg.FT, cfg.NCT, cfg.NJ
    xt = np.ascontiguousarray(xb_perm.T).astype(bf16)
    Wq = W_qkv[:, :NCT * 128]
    Wk = W_qkv[:, Dm:Dm + NCT * 128]
    Wv = W_qkv[:, 2 * Dm:2 * Dm + Hn * 64]
    wq_t = Wq.reshape(FT, 128, NCT, 128).transpose(2, 1, 0, 3)
    wk_t = Wk.reshape(FT, 128, NCT, 128).transpose(2, 1, 0, 3)
    wqk = np.ascontiguousarray(
        np.concatenate([wq_t, wk_t], axis=0)).astype(bf16)
    wv = np.ascontiguousarray(Wv.reshape(FT, 128, Hn * 64)).astype(bf16)
    wfc = np.ascontiguousarray(
        W_fc[:NJ * 128].reshape(NJ, 128, Dm)).astype(bf16)
    bfc = np.ascontiguousarray(
        np.broadcast_to(b_fc.astype(np.float32), (128, Dm)))
    return {"xt": xt, "wqk": wqk, "wv": wv, "wfc": wfc, "bfc": bfc}


_CACHE = {}

SCHEME = "fused+ab4+nf"


def _get_nc(repeats=1):
    key = ("nc", repeats, SCHEME)
    if key not in _CACHE:
        scheme = SCHEME
        fake_exp = scheme.endswith("+fakeexp")
        if fake_exp:
            scheme = scheme[:-len("+fakeexp")]
        norm_fast = scheme.endswith("+nf")
        if norm_fast:
            scheme = scheme[:-len("+nf")]
        abufs = 3
        if "+ab" in scheme:
            scheme, ab = scheme.split("+ab")
            abufs = int(ab)
        nm = "bf16out" if (scheme == "fused3" or norm_fast) else "direct"
        _CACHE[key] = build_nc(Cfg(R=S, Q=S // 2, Hn=H, D=D, repeats=repeats,
                                   attn_scheme=scheme, norm_mode=nm,
                                   abufs=abufs, fake_exp=fake_exp))
    return _CACHE[key]


def make_in_maps(x, W_qkv, W_fc, b_fc):
    cfg = Cfg(R=S, Q=S // 2, Hn=H, D=D)
    x = np.asarray(x, dtype=np.float32)
    in_maps = []
    for c in range(8):
        b, half = divmod(c, 2)
        r0 = half * (S // 2)
        order = np.concatenate([
            np.arange(r0, r0 + S // 2),
            np.arange(0, r0),
            np.arange(r0 + S // 2, S),
        ])
        xb = x[b][order]
        in_maps.append(prep_core_inputs(
            cfg, xb, np.asarray(W_qkv, np.float32),
            np.asarray(W_fc, np.float32), np.asarray(b_fc, np.float32)))
    return in_maps


def kernel(x, char_ids, seq_len, mask, W_qkv, W_fc, b_fc):
    """Full inputs in, full [B, S, D] float32 output out."""
    import os
    # the axon NTFF trace hook is unavailable in this container; make sure
    # an inherited BASS_TRACE=1 cannot send us down that (crashing) path
    os.environ["BASS_NEVER_TRACE"] = "1"
    nc = _get_nc(repeats=1)
    in_maps = make_in_maps(x, W_qkv, W_fc, b_fc)
    res = run_bass_kernel_spmd(nc, in_maps, core_ids=list(range(8)))
    out = np.empty((B, S, D), dtype=np.float32)
    for c in range(8):
        b, half = divmod(c, 2)
        r0 = half * (S // 2)
        out[b, r0:r0 + S // 2, :] = res.results[c]["y"]
    return out

